# revision 23
# baseline (speedup 1.0000x reference)
"""Multi-head attention forward, sharded over 8 NeuronCores.

Sharding: batch (2) x head-group (4 groups of 4 heads) = 8 cores.
Per core (batch b, 4 heads):
  Q^T/K^T = W^T-slices @ x^T (Q bias added on the Act-engine PSUM->SBUF
  copy as a per-partition bias; K bias dropped -- it cancels in softmax;
  V bias folded into out_b on the host), V token-major.
  S^T[k,q] = K^T.T@Q^T per k-tile (scores transposed so the key-padding
  mask is a per-partition activation bias), P^T = exp(scale*S^T + maskbias).
  PV is query-major: ctx2[q,0:65] += P^T[:,q-tile].T @ [V|1] -- full
  128-wide output rows (vs 65/128 columns d-major), ones column gives the
  softmax denominators.  Normalize with a per-partition reciprocal
  multiply (Pool engine), PE-transpose back to d-major ctxT, then
  out_partial = ctx^T.T @ W_o^T-slice -> [2048, 1024] fp32.
Host sums the 4 partials per batch and adds out_b' = out_b + W_o @ b_v.

Cross-rep software pipeline: activation tiles (q/k/v/ctxT) are
double-buffered by rep parity and rep r+1's QKV-projection chunk jobs are
interleaved into rep r's Act-bound attention phase, so the PE-only QKV
phase disappears from the steady-state critical path.
"""

import os
import sys

if "/opt/trn_rl_repo" not in sys.path:
    sys.path.insert(0, "/opt/trn_rl_repo")

import numpy as np
import ml_dtypes

import concourse.bass as bass
import concourse.mybir as mybir
from concourse import bacc
from concourse.bass import ts, ds
from concourse.tile import TileContext
from concourse import bass_utils

BF16 = mybir.dt.bfloat16
F32 = mybir.dt.float32
EXP = mybir.ActivationFunctionType.Exp
MULT = mybir.AluOpType.mult

N_CORES = 8
S = 2048          # sequence length (one batch per core)
HID = 1024
DH = 256          # head dims per core (4 heads x 64)
D = 64
NEG = -50.0       # additive mask bias (post-scale); exp(-50) ~ 2e-22
KC = 1152         # compacted+padded key length; ~1024 unmasked
NKT = KC // 128


def build_program(reps=1):
    nc = bacc.Bacc("TRN2", target_bir_lowering=False, debug=False,
                   num_devices=N_CORES)
    xT = nc.dram_tensor("xT", [HID, S], BF16, kind="ExternalInput").ap()
    xTk = nc.dram_tensor("xTk", [HID, KC], BF16, kind="ExternalInput").ap()
    wqT = nc.dram_tensor("wqT", [HID, DH], BF16, kind="ExternalInput").ap()
    wkT = nc.dram_tensor("wkT", [HID, DH], BF16, kind="ExternalInput").ap()
    wvT = nc.dram_tensor("wvT", [HID, DH], BF16, kind="ExternalInput").ap()
    qbT = nc.dram_tensor("qbT", [128, 2], F32, kind="ExternalInput").ap()
    woT = nc.dram_tensor("woT", [DH, HID], BF16, kind="ExternalInput").ap()
    maskb = nc.dram_tensor("maskb", [128, NKT], F32, kind="ExternalInput").ap()
    out = nc.dram_tensor("out", [S, HID], F32, kind="ExternalOutput").ap()

    pss_bufs = int(os.environ.get("PSS_BUFS", "3"))
    psx_bufs = int(os.environ.get("PSX_BUFS", "2"))
    pt_bufs = int(os.environ.get("PT_BUFS", str(2 * NKT + 2)))
    qkv_every = int(os.environ.get("QKV_EVERY", "3"))
    nbuf = 2 if reps > 1 else 1    # rep-parity double buffering

    with TileContext(nc) as tc:
        with tc.tile_pool(name="const", bufs=1) as cp:
            # weights for K first (first QKV job), then x chunks
            # interleaved on both DMA queues (SP + Act) so the first
            # projection matmuls start after ~2 chunks arrive.
            wk_sb = cp.tile([128, 8, DH], BF16, name="wk_sb")
            nc.sync.dma_start(wk_sb, wkT.rearrange("(c p) m -> p c m", p=128))
            wq_sb = cp.tile([128, 8, DH], BF16, name="wq_sb")
            nc.scalar.dma_start(wq_sb, wqT.rearrange("(c p) m -> p c m",
                                                     p=128))
            qb_sb = cp.tile([128, 2], F32, name="qb_sb")
            nc.scalar.dma_start(qb_sb, qbT)
            maskb_sb = cp.tile([128, NKT], F32, name="maskb_sb")
            nc.scalar.dma_start(maskb_sb, maskb)

            xt_sb = cp.tile([128, 8, S], BF16, name="xt_sb")
            xt_view = xT.rearrange("(c p) t -> c p t", p=128)
            xtk_sb = cp.tile([128, 8, KC], BF16, name="xtk_sb")
            xtk_view = xTk.rearrange("(c p) t -> c p t", p=128)
            for c in range(8):
                nc.sync.dma_start(xtk_sb[:, c, :], xtk_view[c])
                nc.scalar.dma_start(xt_sb[:, c, :], xt_view[c])
            wv_sb = cp.tile([128, 8, DH], BF16, name="wv_sb")
            nc.sync.dma_start(wv_sb, wvT.rearrange("(c p) m -> p c m", p=128))
            wo_sb = cp.tile([128, 2, HID], BF16, name="wo_sb")
            nc.scalar.dma_start(wo_sb, woT.rearrange("(c p) o -> p c o",
                                                     p=128))

            q_sb = [[cp.tile([128, S], BF16, name=f"q_sb{j}_{p}")
                     for j in range(2)] for p in range(nbuf)]
            k_sb = [[cp.tile([128, KC], BF16, name=f"k_sb{j}_{p}")
                     for j in range(2)] for p in range(nbuf)]
            v_sb = [cp.tile([128, NKT, 4, 65], BF16, name=f"v_sb_{p}")
                    for p in range(nbuf)]
            ctxT = [[cp.tile([128, S], BF16, name=f"ctxT{j}_{p}")
                     for j in range(2)] for p in range(nbuf)]
            for p in range(nbuf):
                nc.vector.memset(v_sb[p][:, :, :, 64:65], 1.0)

            with tc.tile_pool(name="psS", bufs=pss_bufs,
                              space="PSUM") as psS, \
                 tc.tile_pool(name="psX", bufs=psx_bufs,
                              space="PSUM") as psX, \
                 tc.tile_pool(name="ptp", bufs=pt_bufs) as ptp, \
                 tc.tile_pool(name="npool", bufs=8) as npool, \
                 tc.tile_pool(name="outp", bufs=3) as outp:

                from collections import deque

                def make_qkv_jobs(p):
                    """Chunk-granular QKV projection jobs for parity p.
                    Order: K0, Q0, V, K1, Q1 (attention consumes pair 0
                    first; PV units consume V tiles)."""
                    jobs = []

                    def qk_chunk(kind, j, off, w):
                        def go():
                            ps = psS.tile([128, 512], F32, name="ps_qkv",
                                          tag="s_ps")
                            src = xt_sb if kind == "q" else xtk_sb
                            wt = wq_sb if kind == "q" else wk_sb
                            for c in range(8):
                                nc.tensor.matmul(
                                    ps[:, 0:w],
                                    lhsT=wt[:, c, ts(j, 128)],
                                    rhs=src[:, c, ds(off, w)],
                                    start=(c == 0), stop=(c == 7))
                            if kind == "q":
                                nc.scalar.add(q_sb[p][j][:, ds(off, w)],
                                              ps[:, 0:w], qb_sb[:, j:j + 1])
                            else:
                                nc.vector.tensor_copy(
                                    k_sb[p][j][:, ds(off, w)], ps[:, 0:w])
                        return go

                    def v_tile(i):
                        def go():
                            ps = psS.tile([128, 4, 64], F32, name="ps_v",
                                          tag="s_ps")
                            for c in range(8):
                                nc.tensor.matmul(ps[:, :, :],
                                                 lhsT=xtk_sb[:, c, ts(i, 128)],
                                                 rhs=wv_sb[:, c, :],
                                                 start=(c == 0), stop=(c == 7))
                            nc.vector.tensor_copy(v_sb[p][:, i, :, 0:64],
                                                  ps[:, :, :])
                        return go

                    def kchunks():
                        off = 0
                        while off < KC:
                            w = min(512, KC - off)
                            yield off, w
                            off += w

                    for off, w in kchunks():
                        jobs.append(qk_chunk("k", 0, off, w))
                    for n in range(4):
                        jobs.append(qk_chunk("q", 0, n * 512, 512))
                    for i in range(NKT):
                        jobs.append(v_tile(i))
                    for off, w in kchunks():
                        jobs.append(qk_chunk("k", 1, off, w))
                    for n in range(4):
                        jobs.append(qk_chunk("q", 1, n * 512, 512))
                    return jobs

                deferred = deque()      # outproj unit queue (ti, parity)
                next_jobs = deque()     # next rep's QKV chunk jobs

                def emit_outproj_unit(ti, p):
                    o_sb = outp.tile([128, HID], F32, name="o_sb")
                    for oc in range(2):
                        o_ps = psS.tile([128, 512], F32, name="o_ps",
                                        tag="s_ps")
                        for hc in range(2):
                            nc.tensor.matmul(
                                o_ps, lhsT=ctxT[p][hc][:, ts(ti, 128)],
                                rhs=wo_sb[:, hc, ts(oc, 512)],
                                start=(hc == 0), stop=(hc == 1))
                        nc.vector.tensor_copy(o_sb[:, ts(oc, 512)], o_ps)
                    nc.sync.dma_start(out[ts(ti, 128)], o_sb)

                cur_cnw = [None]

                def emit_pv_unit(p, jqc, jpr, qt, hi, pts):
                    ctx2 = psX.tile([128, 65], F32, name="ctx_ps", tag="x")
                    for kt in range(NKT):
                        nc.tensor.matmul(
                            ctx2, lhsT=pts[kt][:, ds(hi * 512 + qt * 128,
                                                     128)],
                            rhs=v_sb[p][:, kt, 2 * jpr + hi, :],
                            start=(kt == 0), stop=(kt == NKT - 1))
                    r = npool.tile([128, 1], F32, name="r")
                    nc.vector.reciprocal(r, ctx2[:, 64:65])
                    # pair both heads of this q-tile in one [q, 2*64] tile,
                    # then one SBUF->SBUF DMA-transpose writes the d-major
                    # [128,128] ctxT block (no PE transpose, no psum)
                    if hi == 0:
                        cur_cnw[0] = npool.tile([128, 2, 64], BF16,
                                                name="cnw")
                    nc.vector.tensor_scalar_mul(cur_cnw[0][:, hi, :],
                                                ctx2[:, 0:64], r)
                    if hi == 1:
                        nc.sync.dma_start(
                            ctxT[p][jpr][:, ds(jqc * 512 + qt * 128, 128)],
                            cur_cnw[0], transpose=True)

                for rep in range(reps):
                    p = rep % nbuf
                    if rep == 0:
                        for job in make_qkv_jobs(0):
                            job()
                    if rep + 1 < reps:
                        next_jobs.extend(make_qkv_jobs((rep + 1) % nbuf))

                    prev_units = []
                    slot = 0
                    for qc in range(4):          # 512-token query chunks
                        for pr in range(2):      # head pairs (2pr, 2pr+1)
                            pts = []
                            for kt in range(NKT):
                                s_ps = psS.tile([128, 1024], F32, name="s_ps",
                                                tag="s_ps")
                                for col in range(2):
                                    hr = col * 64
                                    nc.tensor.matmul(
                                        s_ps[:, ts(col, 512)],
                                        lhsT=k_sb[p][pr][hr:hr + 64,
                                                         ts(kt, 128)],
                                        rhs=q_sb[p][pr][hr:hr + 64,
                                                        ds(qc * 512, 512)],
                                        start=True, stop=True,
                                        tile_position=(hr, 0))
                                pt = ptp.tile([128, 1024], BF16, name="pt")
                                nc.scalar.activation(pt, s_ps, EXP,
                                                     bias=maskb_sb[:,
                                                                   kt:kt + 1],
                                                     scale=0.125)
                                pts.append(pt)
                                # interleave prev pair's PV units, next
                                # rep's QKV chunks, and outproj with this
                                # pair's score/exp stream
                                if slot % 2 == 1 and prev_units:
                                    emit_pv_unit(*prev_units.pop(0))
                                if slot % qkv_every == 0 and next_jobs:
                                    next_jobs.popleft()()
                                if deferred and slot % 3 == 2:
                                    emit_outproj_unit(*deferred.popleft())
                                slot += 1
                            while prev_units:
                                emit_pv_unit(*prev_units.pop(0))
                            prev_units = [(p, qc, pr, qt, hi, pts)
                                          for qt in range(4)
                                          for hi in range(2)]
                            if pr == 0 and qc >= 1:
                                deferred.extend((4 * (qc - 1) + i, p)
                                                for i in range(4))
                    # per-rep tail: last pair's PV units + last chunk's
                    # outproj carry into the next rep's attention stream.
                    while prev_units:
                        emit_pv_unit(*prev_units.pop(0))
                    deferred.extend((12 + i, p) for i in range(4))
                    if rep == reps - 1:
                        while next_jobs:
                            next_jobs.popleft()()
                        while deferred:
                            emit_outproj_unit(*deferred.popleft())

    nc.compile()
    return nc


_NC = None


def shard_inputs(x, mask, qkv_w, qkv_b, out_w):
    bf = ml_dtypes.bfloat16
    in_maps = []
    for c in range(N_CORES):
        b, g = c // 4, c % 4
        hs = slice(DH * g, DH * (g + 1))
        xTc = np.ascontiguousarray(x[b].T).astype(bf)
        idx = np.where(mask[b] != 0)[0]
        assert len(idx) <= KC, f"unmasked {len(idx)} > KC={KC}"
        pad = np.zeros(KC - len(idx), np.int64)
        idxp = np.concatenate([idx, pad])
        xTkc = np.ascontiguousarray(x[b][idxp].T).astype(bf)
        mbk = np.full(KC, np.float32(NEG), np.float32)
        mbk[:len(idx)] = 0.0
        mbk = np.ascontiguousarray(mbk.reshape(NKT, 128).T).astype(np.float32)
        wq = np.ascontiguousarray(qkv_w[hs, :].T).astype(bf)
        wk = np.ascontiguousarray(qkv_w[1024 + DH * g:1024 + DH * (g + 1), :].T
                                  ).astype(bf)
        wv = np.ascontiguousarray(qkv_w[2048 + DH * g:2048 + DH * (g + 1), :].T
                                  ).astype(bf)
        qb = np.ascontiguousarray(
            qkv_b[hs].reshape(2, 128).T).astype(np.float32)
        wo = np.ascontiguousarray(out_w[:, hs].T).astype(bf)
        in_maps.append({"xT": xTc, "xTk": xTkc, "wqT": wq, "wkT": wk,
                        "wvT": wv, "qbT": qb, "woT": wo, "maskb": mbk})
    return in_maps


def run(in_maps, **kwargs):
    global _NC
    if _NC is None:
        _NC = build_program()
    return bass_utils.run_bass_kernel_spmd(
        _NC, in_maps, core_ids=list(range(N_CORES)), **kwargs)


def kernel(x, mask, qkv_w, qkv_b, out_w, out_b):
    global KC, NKT, _NC
    x = np.asarray(x)
    mask = np.asarray(mask)
    need = int(np.max(np.sum(mask != 0, axis=1)))
    kc = max(128, ((need + 127) // 128) * 128)
    if kc != KC:
        KC, NKT = kc, kc // 128
        _NC = None
    qkv_w = np.asarray(qkv_w)
    qkv_b = np.asarray(qkv_b)
    out_w = np.asarray(out_w)
    out_b = np.asarray(out_b)
    in_maps = shard_inputs(x, mask, qkv_w, qkv_b, out_w)
    res = run(in_maps)
    parts = [r["out"] for r in res.results]
    # V bias folded: (ctx + b_v) @ W_o^T = ctx @ W_o^T + W_o b_v
    out_b_adj = out_b + out_w @ qkv_b[2048:3072]
    full = np.empty((2, S, HID), np.float32)
    for b in range(2):
        acc = parts[4 * b].astype(np.float32)
        for g in range(1, 4):
            acc = acc + parts[4 * b + g]
        full[b] = acc + out_b_adj[None, :]
    return full
